# revision 14
# baseline (speedup 1.0000x reference)
"""Cross-attention layer kernel for 8 Trainium2 NeuronCores.

Reference computation (fp32, D=1024, S=2048, B=4):
    q = x @ Wq.T + bq ; k = x @ Wk.T + bk ; v = x @ Wv.T + bv
    attn = softmax(q @ k.T / 32)
    vision = attn @ v                      # [B,S,D]
    text   = attn.T @ x                    # [B,S,D]

Sharding: core c handles batch b=c//2, sequence-half h=c%2 (1024 rows).
Those rows are both the core's queries and its share of the keys, so each
core projects Q/K/V only for its own 1024 rows and the K^T / V halves are
pair-AllGathered (replica groups [2b, 2b+1]) through DRAM bounce buffers
into full, globally-ordered SBUF copies.  Everything else is local: scores
for all 2048 keys, exp (softmax without max-subtraction; scores are
bounded ~1.7), P^T transposes, vision, and a textT partial contracted
over own queries that the host sums across the pair.

All storage is bf16 (matmuls run 1 cycle/row, PSUM accumulates fp32),
which halves DMA and lets every operand stay SBUF-resident - no spills.
x^T comes from hardware DMA-transposes (xbar), not the PE.  The K
projection + AllGather run first (split in two so the exchange starts
as soon as half of K^T is projected), then V + its AllGather, then Q,
so both collectives hide behind ~50us of projection compute before the
scores need K and C2 needs V.  The 1/rowsum softmax normalization is
folded into both outputs.

Per-core PE work: 3x65K projection cycles + 131K scores + 16K P^T
transposes + 131K vision + 131K textT ~= 604K cycles ~= 252us at 2.4GHz.
"""

import sys

import numpy as np
import ml_dtypes

try:
    import concourse.bass as bass
except ImportError:  # pragma: no cover - grading env should have it on path
    sys.path.insert(0, "/opt/trn_rl_repo")
    import concourse.bass as bass

import concourse.mybir as mybir
import concourse.tile as tile
from concourse import bacc
from concourse.bass_utils import run_bass_kernel_spmd
from concourse.masks import make_identity

F32 = mybir.dt.float32
BF16 = mybir.dt.bfloat16
BF16_NP = ml_dtypes.bfloat16

B = 4          # batches
S = 2048       # sequence length
D = 1024       # model dim
SH = S // 2    # rows per core (own queries == own keys)
P = 128        # partitions
NT = D // P    # 8 tiles along d/e
NQ = SH // P   # 8 q-tiles per core
NKH = SH // P  # 8 own-half k-tiles
NK = S // P    # 16 k-tiles total
NC = S // 512  # 4 512-chunks along global k
SCALE = 1.0 / 32.0  # 1/sqrt(D)
N512 = 512
GROUPS = [[0, 1], [2, 3], [4, 5], [6, 7]]


def build_program():
    nc = bacc.Bacc("TRN2", target_bir_lowering=False, debug=False, num_devices=8)

    xq_h = nc.dram_tensor("xq", [SH, D], BF16, kind="ExternalInput")
    wqt_h = nc.dram_tensor("wqt", [D, D], BF16, kind="ExternalInput")
    wkt_h = nc.dram_tensor("wkt", [D, D], BF16, kind="ExternalInput")
    wvt_h = nc.dram_tensor("wvt", [D, D], BF16, kind="ExternalInput")
    bq_h = nc.dram_tensor("bq", [D], F32, kind="ExternalInput")
    bk_h = nc.dram_tensor("bk", [D], F32, kind="ExternalInput")
    bv_h = nc.dram_tensor("bv", [D], F32, kind="ExternalInput")

    vision_h = nc.dram_tensor("vision", [SH, D], BF16, kind="ExternalOutput")
    textT_h = nc.dram_tensor("textT", [D, S], BF16, kind="ExternalOutput")

    # tiled DRAM views
    xq_ap = xq_h.ap()                                        # [1024,1024]
    xq_pid = xq_h.ap().rearrange("(i p) d -> p i d", p=P)    # [128,8,1024]
    wq_r = wqt_h.ap().rearrange("(t p) e -> p t e", p=P)     # [128,8,1024]
    wk_r = wkt_h.ap().rearrange("(t p) e -> p t e", p=P)
    wv_r = wvt_h.ap().rearrange("(t p) e -> p t e", p=P)
    bq_r = bq_h.ap().rearrange("(t p) -> p t", p=P)          # [128,8]
    bk_r = bk_h.ap().rearrange("(t p) -> p t", p=P)

    bv_ap = bv_h.ap()
    bv_bcast_src = bass.AP(tensor=bv_ap.tensor, offset=bv_ap.offset,
                           ap=[[0, P], bv_ap.ap[0]])         # [128,1024] bcast

    with tile.TileContext(nc) as tc:
        with (
            tc.tile_pool(name="singles", bufs=1) as singles,
            tc.tile_pool(name="dram", bufs=1, space="DRAM") as dram_pool,
        ):
            # DRAM bounce buffers for the pair AllGathers
            kh0_d = dram_pool.tile([D // 2, SH], BF16)  # own K^T rows e<512
            kh1_d = dram_pool.tile([D // 2, SH], BF16)  # own K^T rows e>=512
            vh_d = dram_pool.tile([SH, D], BF16)        # own V half [k, e]
            kg0_d = dram_pool.tile([2, D // 2, SH], BF16)
            kg1_d = dram_pool.tile([2, D // 2, SH], BF16)
            vg_d = dram_pool.tile([2, SH, D], BF16)     # gathered V
            kg0_r = kg0_d.rearrange("h (t p) k -> p h t k", p=P)  # [128,2,4,1024]
            kg1_r = kg1_d.rearrange("h (t p) k -> p h t k", p=P)
            vg_r = vg_d.rearrange("h (i p) e -> p h i e", p=P)    # [128,2,8,1024]

            ident_f = singles.tile([P, P], F32)
            make_identity(nc, ident_f)
            ident = singles.tile([P, P], BF16)
            nc.vector.tensor_copy(ident, ident_f)
            bq_sb = singles.tile([P, NT], F32)
            nc.sync.dma_start(out=bq_sb, in_=bq_r)
            bk_sb = singles.tile([P, NT], F32)
            nc.sync.dma_start(out=bk_sb, in_=bk_r)
            bvb = singles.tile([P, D], F32)
            nc.sync.dma_start(out=bvb, in_=bv_bcast_src)
            r_all = singles.tile([P, NQ], F32)

            # whole-kernel resident tensors
            xq_sb = singles.tile([P, NQ, D], BF16)       # own rows, natural
            qt = singles.tile([P, NT, SH], BF16)         # Q^T [e, q]
            kT = singles.tile([P, NT, S], BF16)          # K^T [e, k global]
            v_sb = singles.tile([P, NK, D], BF16)        # V [k global, e]
            P_sb = singles.tile([P, NQ, S], BF16)        # exp(scores) [q, k]

            with (
                tc.tile_pool(name="wpool", bufs=3) as wpool,
                tc.tile_pool(name="xtpool", bufs=1) as xtpool,
                tc.tile_pool(name="stg", bufs=4) as stg,
                tc.tile_pool(name="mmps", bufs=4, space="PSUM") as mmps,
            ):
                xqT = xtpool.tile([P, NT, SH], BF16, tag="xqT")  # x^T [d, q]

                def w_half(src_r, h):
                    wt = wpool.tile([P, NT, N512], BF16, tag="wh", name="wt")
                    nc.gpsimd.dma_start(
                        out=wt, in_=src_r[:, :, h * N512:(h + 1) * N512])
                    return wt

                # ---- phase A: hardware DMA-transposes of own rows -------
                # xqT[p, t, q] = xq[q, 128t+p]; first-half q columns land
                # first so the K projection can start early.
                wk0 = w_half(wk_r, 0)
                wk1 = w_half(wk_r, 1)
                for qh in range(2):
                    for t in range(NT):
                        eng = (nc.sync, nc.scalar)[t % 2]
                        eng.dma_start_transpose(
                            out=xqT[:, t, qh * N512:(qh + 1) * N512],
                            in_=xq_ap[qh * N512:(qh + 1) * N512,
                                      t * P:(t + 1) * P])
                # natural-layout copy (only needed from phase C2 on)
                for g in range(2):
                    nc.gpsimd.dma_start(
                        out=xq_sb[:, 4 * g:4 * g + 4, :],
                        in_=xq_pid[:, 4 * g:4 * g + 4, :])

                # ---- phase B1: K^T projection + split AllGather ---------
                for h, wt in ((0, wk0), (1, wk1)):
                    kh_d = (kh0_d, kh1_d)[h]
                    for tl in range(4):
                        te = h * 4 + tl
                        ev = stg.tile([P, 2 * N512], BF16, tag="ev")
                        for kc in range(2):
                            ps = mmps.tile([P, N512], F32, tag="acc")
                            for td in range(NT):
                                nc.tensor.matmul(
                                    ps,
                                    wt[:, td, tl * P:(tl + 1) * P],
                                    xqT[:, td, kc * N512:(kc + 1) * N512],
                                    start=(td == 0), stop=(td == NT - 1))
                            nc.scalar.activation(
                                ev[:, kc * N512:(kc + 1) * N512], ps,
                                mybir.ActivationFunctionType.Identity,
                                bias=bk_sb[:, te:te + 1], scale=1.0)
                        eng = (nc.sync, nc.scalar)[tl % 2]
                        eng.dma_start(
                            out=kh_d[tl * P:(tl + 1) * P, :], in_=ev)
                    nc.gpsimd.collective_compute(
                        "AllGather", mybir.AluOpType.bypass,
                        replica_groups=GROUPS,
                        ins=[(kh0_d, kh1_d)[h].opt()],
                        outs=[(kg0_d, kg1_d)[h].opt()])
                    # gathered K^T rows -> SBUF, global key order
                    kg_r = (kg0_r, kg1_r)[h]
                    for hh in range(2):
                        eng = (nc.sync, nc.scalar)[hh]
                        eng.dma_start(
                            out=kT[:, 4 * h:4 * h + 4, hh * SH:(hh + 1) * SH],
                            in_=kg_r[:, hh, :, :])

                # ---- phase B2: V projection (own keys) + AllGather ------
                wv0 = w_half(wv_r, 0)
                wv1 = w_half(wv_r, 1)
                for ki in range(NKH):
                    ev = stg.tile([P, 2 * N512], BF16, tag="ev")
                    for h, wt in ((0, wv0), (1, wv1)):
                        ps = mmps.tile([P, N512], F32, tag="acc")
                        for td in range(NT):
                            nc.tensor.matmul(
                                ps,
                                xqT[:, td, ki * P:(ki + 1) * P],
                                wt[:, td, :],
                                start=(td == 0), stop=(td == NT - 1))
                        nc.vector.tensor_add(
                            ev[:, h * N512:(h + 1) * N512], ps,
                            bvb[:, h * N512:(h + 1) * N512])
                    eng = (nc.sync, nc.scalar)[ki % 2]
                    eng.dma_start(out=vh_d[ki * P:(ki + 1) * P, :], in_=ev)
                nc.gpsimd.collective_compute(
                    "AllGather", mybir.AluOpType.bypass,
                    replica_groups=GROUPS,
                    ins=[vh_d.opt()], outs=[vg_d.opt()])
                for hh in range(2):
                    eng = (nc.sync, nc.scalar)[hh]
                    eng.dma_start(
                        out=v_sb[:, hh * NKH:(hh + 1) * NKH, :],
                        in_=vg_r[:, hh, :, :])

                # ---- phase B3: Q^T projection (own queries, resident) ---
                for h in range(2):
                    wt = w_half(wq_r, h)
                    for tl in range(4):
                        te = h * 4 + tl
                        for qc in range(2):
                            ps = mmps.tile([P, N512], F32, tag="acc")
                            for td in range(NT):
                                nc.tensor.matmul(
                                    ps,
                                    wt[:, td, tl * P:(tl + 1) * P],
                                    xqT[:, td, qc * N512:(qc + 1) * N512],
                                    start=(td == 0), stop=(td == NT - 1))
                            nc.scalar.activation(
                                qt[:, te, qc * N512:(qc + 1) * N512], ps,
                                mybir.ActivationFunctionType.Identity,
                                bias=bq_sb[:, te:te + 1], scale=1.0)

                # ---- phase C1: scores + exp(+rowsum); P resident --------
                with tc.tile_pool(name="phC1_l", bufs=4) as phC1_l:
                    for j in range(NQ):
                        l4 = phC1_l.tile([P, NC], F32, tag="l4")
                        for kc in range(NC):
                            ps = mmps.tile([P, N512], F32, tag="acc")
                            for t in range(NT):
                                nc.tensor.matmul(
                                    ps,
                                    qt[:, t, j * P:(j + 1) * P],
                                    kT[:, t, kc * N512:(kc + 1) * N512],
                                    start=(t == 0), stop=(t == NT - 1))
                            nc.scalar.activation(
                                P_sb[:, j, kc * N512:(kc + 1) * N512], ps,
                                mybir.ActivationFunctionType.Exp,
                                bias=0.0, scale=SCALE,
                                accum_out=l4[:, kc:kc + 1])
                        lsum = phC1_l.tile([P, 1], F32, tag="lsum")
                        nc.vector.reduce_sum(out=lsum, in_=l4,
                                             axis=mybir.AxisListType.X)
                        nc.vector.reciprocal(out=r_all[:, j:j + 1], in_=lsum)

            # ---- phase C2: P^T transposes + vision (pipelined) ----------
            with (
                tc.tile_pool(name="phD_xs", bufs=1) as phD_xs,
                tc.tile_pool(name="phC2_pt", bufs=2) as phC2_pt,
                tc.tile_pool(name="phC2_ev", bufs=3) as phC2_ev,
            ):
                # scale phase D's x_q while C2 computes
                xs = phD_xs.tile([P, NQ, D], BF16, tag="xs")
                for j in range(NQ):
                    nc.vector.tensor_scalar_mul(
                        xs[:, j, :], xq_sb[:, j, :], r_all[:, j:j + 1])

                with (
                    tc.tile_pool(name="phC2_tr", bufs=2,
                                 space="PSUM") as phC2_tr,
                    tc.tile_pool(name="phC2_vp", bufs=4,
                                 space="PSUM") as phC2_vp,
                ):
                    def transposes(j):
                        ptj = phC2_pt.tile([P, NK, P], BF16, tag="ptj",
                                           name="ptj")
                        for i in range(NK):
                            ps = phC2_tr.tile([P, P], BF16, tag="tr")
                            nc.tensor.transpose(
                                ps, P_sb[:, j, i * P:(i + 1) * P], ident)
                            nc.vector.tensor_copy(out=ptj[:, i, :], in_=ps)
                        return ptj

                    def vision(j, ptj):
                        ev = phC2_ev.tile([P, D], BF16, tag="ev")
                        for h in range(2):
                            ps = phC2_vp.tile([P, N512], F32, tag="vp")
                            for i in range(NK):
                                nc.tensor.matmul(
                                    ps,
                                    ptj[:, i, :],
                                    v_sb[:, i, h * N512:(h + 1) * N512],
                                    start=(i == 0), stop=(i == NK - 1))
                            nc.vector.tensor_scalar_mul(
                                ev[:, h * N512:(h + 1) * N512], ps,
                                r_all[:, j:j + 1])
                        eng = (nc.sync, nc.scalar, nc.gpsimd)[j % 3]
                        eng.dma_start(
                            out=vision_h.ap()[j * P:(j + 1) * P, :], in_=ev)

                    prev = transposes(0)
                    for j in range(1, NQ):
                        cur = transposes(j)
                        vision(j - 1, prev)
                        prev = cur
                    vision(NQ - 1, prev)

                # ---- phase D: textT = (x_q * r).T @ P -------------------
                with (
                    tc.tile_pool(name="phD_ev", bufs=3) as phD_ev,
                    tc.tile_pool(name="phD_ps", bufs=8, space="PSUM") as phD_ps,
                ):
                    for dc in range(NT):
                        ev = phD_ev.tile([P, S], BF16, tag="ev")
                        for kc in range(NC):
                            ps = phD_ps.tile([P, N512], F32, tag="tp")
                            for j in range(NQ):
                                nc.tensor.matmul(
                                    ps,
                                    xs[:, j, dc * P:(dc + 1) * P],
                                    P_sb[:, j, kc * N512:(kc + 1) * N512],
                                    start=(j == 0), stop=(j == NQ - 1))
                            nc.scalar.copy(
                                out=ev[:, kc * N512:(kc + 1) * N512], in_=ps)
                        eng = (nc.sync, nc.scalar, nc.gpsimd)[dc % 3]
                        eng.dma_start(
                            out=textT_h.ap()[dc * P:(dc + 1) * P, :], in_=ev)

    nc.compile()
    return nc


_NC_CACHE = []


def _get_program():
    if not _NC_CACHE:
        _NC_CACHE.append(build_program())
    return _NC_CACHE[0]


def kernel(inputs, Wq, bq, Wk, bk, Wv, bv, _run_opts=None):
    x = np.asarray(inputs, dtype=np.float32).astype(BF16_NP)
    WqT = np.ascontiguousarray(np.asarray(Wq, dtype=np.float32).T).astype(BF16_NP)
    WkT = np.ascontiguousarray(np.asarray(Wk, dtype=np.float32).T).astype(BF16_NP)
    WvT = np.ascontiguousarray(np.asarray(Wv, dtype=np.float32).T).astype(BF16_NP)
    bq = np.ascontiguousarray(np.asarray(bq, dtype=np.float32))
    bk = np.ascontiguousarray(np.asarray(bk, dtype=np.float32))
    bv = np.ascontiguousarray(np.asarray(bv, dtype=np.float32))

    nc = _get_program()

    in_maps = []
    for c in range(8):
        b, h = divmod(c, 2)
        xq = np.ascontiguousarray(x[b, h * SH:(h + 1) * SH])
        in_maps.append({
            "xq": xq,
            "wqt": WqT, "wkt": WkT, "wvt": WvT,
            "bq": bq, "bk": bk, "bv": bv,
        })

    run_opts = dict(_run_opts or {})
    res = run_bass_kernel_spmd(nc, in_maps, core_ids=list(range(8)), **run_opts)
    results = res.results

    vision = np.empty((B, S, D), np.float32)
    text = np.empty((B, S, D), np.float32)
    for b in range(B):
        for h in range(2):
            vision[b, h * SH:(h + 1) * SH] = \
                results[2 * b + h]["vision"].astype(np.float32)
        tT = (results[2 * b]["textT"].astype(np.float32)
              + results[2 * b + 1]["textT"].astype(np.float32))
        text[b] = tT.T
    if _run_opts is not None:
        return (vision, text), res
    return (vision, text)
